# revision 35
# baseline (speedup 1.0000x reference)
"""Trainium2 Bass kernel for nn_DilatedKnnGraph (dilated knn edge list).

Problem: x is (65536, 256) fp32 = 64 strokes x 1024 points x 256 dims.
Per stroke: pairwise sq-distance matrix (1024x1024), top-18 neighbors per
point (k=9, dilation=2), edge list (2, S*L*18) sliced [::2] ->
output (2, 589824) int32: row0 = neighbor indices at even ranks
0,2,...,16; row1 = center index repeated 9x.

Sharding: data parallel over strokes; each of the 8 cores handles 8
strokes and emits its contiguous (2, 73728) slice of the edge list.

Per-core device algorithm (per stroke s, row-tile t of 128 points):
 - XT (256x1024, feature-major, 2 chunks of 128 partitions) via PE
   transposes of the naturally-loaded X tiles.
 - sq[m] = ||x_m||^2 via ACT Square with accumulator per X tile; the
   per-partition columns are PE-transposed into a free-dim row, scaled
   by -0.5 (ACT), and decomposed into three bf16 rows hi/mid/lo
   (|error| ~2e-6) so the bias rides a K=3 bf16 matmul.
 - V0 psum tile (128,1024) accumulates, all on PE: bias matmul
   (lhsT=ones(3,128), rhs=bias rows) + 2 fp32 K=128 Gram matmuls
   (lhsT = XT m-tile, rhs = XT) + (-1e6*I) @ I on the self-distance
   diagonal block.  Row ordering of V0 equals ordering of -distance^2
   (the per-row constant -0.5*sq_l is dropped; the self column is
   pushed to -1e6 and re-emitted directly as output column 0).
 - Top-k per row on DVE: max8 -> match_replace(-inf) -> max8 gives the
   values at no-self ranks 0..15; output needs [self] + no-self ranks
   1,3,..,15.  The 8 odd-rank values are packed on ACT and one
   max_index searching V0 yields their column indices with jax-top_k
   tie order (first-unmatched-occurrence semantics).  The max_index is
   software-pipelined one tile behind the max8 scans so neither the
   ACT pack nor the GPSIMD assembly sits on the DVE critical path.
 - GPSIMD assembles (128,9) index blocks (iota for centers, +stroke
   imm and +core-base AP offsets in fp32, exact below 2^24) into a
   per-stroke staging tile, cast to uint32 and DMA'd to the output.
 - row1 (center repeated 9x) is a host-precomputed iota constant DMA'd
   dram->dram.
"""

import ml_dtypes
import numpy as np

import bass_rust
import concourse.bass as bass
import concourse.mybir as mybir
import concourse.tile as tile_mod
from concourse.bass_utils import run_bass_kernel_spmd
from concourse.tile import TileContext
from concourse.tile_rust import add_dep_helper
from concourse.vector_clock import ScopedClock

S, L, D = 64, 1024, 256
N_CORES = 8
S_PER_CORE = S // N_CORES          # 8 strokes per core
KOUT = 9                           # surviving neighbors per point
PTS_PER_CORE = S_PER_CORE * L      # 8192
COLS_PER_CORE = PTS_PER_CORE * KOUT  # 73728
NEG_BIG = -1.0e6                   # self-distance zap
NEG_INF = -3.0e38                  # match_replace fill
F32 = mybir.dt.float32
U32 = mybir.dt.uint32
BF16 = mybir.dt.bfloat16

# ---------------------------------------------------------------------------
# Workaround: the walrus build in this container rejects instructions that
# carry more than a couple of semaphore waits ("Too many sync wait
# commands").  (1) replace TileContext's final Drain (which carries the whole
# global clock) with single-wait SP EventSemaphore nops; (2) post-pass that
# hoists excess waits from any instruction onto same-engine nops.
# ---------------------------------------------------------------------------
_MAX_WAITS = 1
_wsplit_ctr = [0]


def _mk_wait_carrier(engine, waits):
    _wsplit_ctr[0] += 1
    nop = bass_rust.InstEventSemaphore(
        name=f"I-wsplit-{_wsplit_ctr[0]}", ins=[], outs=[]
    )
    nop.engine = engine
    nop.sync_info = bass_rust.SyncInfo(on_wait=list(waits), on_update=[])
    return nop


def _patched_drain_and_barrier(self, tick_clock, wait_clock):
    nc = self.nc
    collector = nc.sync.nop()
    wait_clock.add_sem_waits(
        collector.ins, ScopedClock({None: tick_clock.global_clock})
    )
    si = collector.ins.sync_info
    waits = list(si.on_wait) if (si and si.on_wait) else []
    if len(waits) > _MAX_WAITS:
        si.on_wait = waits[:_MAX_WAITS]
        rest = waits[_MAX_WAITS:]
        for i in range(0, len(rest), _MAX_WAITS):
            chunk = rest[i : i + _MAX_WAITS]
            nop = nc.sync.nop()
            nsi = nop.ins.sync_info
            if nsi is None:
                nop.ins.sync_info = bass_rust.SyncInfo(on_wait=chunk, on_update=[])
            else:
                nsi.on_wait = list(nsi.on_wait or []) + chunk
    nc.sync.drain()

    nc.all_engine_barrier()
    assert self.sems is not None
    popped = nc._tile_sem_poison_stack.pop()
    assert popped is self._sem_poison
    nc.clear_and_free_semaphores(list(self.sems.allocated().values()))
    nc.all_engine_barrier()


tile_mod.TileContext._drain_and_barrier = _patched_drain_and_barrier


# ---------------------------------------------------------------------------
# Workaround 2: bass2jax.run_bass_via_pjrt converts the shard_map output to
# numpy via np.asarray on the GLOBAL sharded array, which makes jax compile a
# cross-device gather HLO through the full neuronx-cc pipeline — and that
# gather module fails codegen here.  Replace with a version that fetches each
# device's shard directly (plain D2H copies, no gather program).
# ---------------------------------------------------------------------------
def _install_pjrt_patch():
    import jax
    from jax.sharding import Mesh, PartitionSpec
    try:
        from jax.experimental.shard_map import shard_map
    except ImportError:
        from jax.shard_map import shard_map  # newer jax
    from concourse import bass2jax as b2j

    if getattr(b2j, "_knn_patch_installed", False):
        return

    def run_bass_via_pjrt(nc, in_maps, n_cores):
        b2j.install_neuronx_cc_hook()
        assert nc.dbg_addr is None, "debug not supported in patched runner"
        partition_name = (
            nc.partition_id_tensor.name if nc.partition_id_tensor else None
        )

        in_names, out_names, out_avals, zero_outs = [], [], [], []
        for alloc in nc.m.functions[0].allocations:
            if not isinstance(alloc, mybir.MemoryLocationSet):
                continue
            name = alloc.memorylocations[0].name
            if alloc.kind == "ExternalInput":
                if name != partition_name:
                    in_names.append(name)
            elif alloc.kind == "ExternalOutput":
                shape = list(alloc.tensor_shape)
                np_dtype = mybir.dt.np(alloc.dtype)
                out_names.append(name)
                out_avals.append(
                    jax.core.ShapedArray(tuple(shape), np_dtype)
                )
                zero_outs.append(np.zeros(shape, np_dtype))

        n_params = len(in_names)
        n_outs = len(out_avals)
        in_names.extend(out_names)
        if partition_name is not None:
            in_names.append(partition_name)

        donate = tuple(range(n_params, n_params + n_outs))

        def _body(*args):
            operands = list(args)
            if partition_name is not None:
                operands.append(b2j.partition_id_tensor())
            outs = b2j._bass_exec_p.bind(
                *operands,
                out_avals=tuple(out_avals),
                in_names=tuple(in_names),
                out_names=tuple(out_names),
                lowering_input_output_aliases=(),
                sim_require_finite=True,
                sim_require_nnan=True,
                nc=nc,
            )
            return tuple(outs)

        devices = jax.devices()[:n_cores]
        assert len(devices) == n_cores
        mesh = Mesh(np.asarray(devices), ("core",))
        in_specs = (PartitionSpec("core"),) * (n_params + n_outs)
        out_specs = (PartitionSpec("core"),) * len(out_names)
        sharded = jax.jit(
            shard_map(
                _body,
                mesh=mesh,
                in_specs=in_specs,
                out_specs=out_specs,
                check_rep=False,
            ),
            donate_argnums=donate,
            keep_unused=True,
        )
        per_core = [
            [np.asarray(m[name]) for name in in_names[:n_params]] for m in in_maps
        ]
        concat_in = [
            np.concatenate([per_core[c][i] for c in range(n_cores)], axis=0)
            for i in range(n_params)
        ]
        concat_zeros = [
            np.zeros((n_cores * z.shape[0], *z.shape[1:]), z.dtype)
            for z in zero_outs
        ]
        out_arrs = sharded(*concat_in, *concat_zeros)

        results = [dict() for _ in range(n_cores)]
        for i, name in enumerate(out_names):
            arr = out_arrs[i]
            shards = sorted(
                arr.addressable_shards, key=lambda s: s.index[0].start or 0
            )
            assert len(shards) == n_cores
            for c, sh in enumerate(shards):
                results[c][name] = np.asarray(sh.data)
        return results

    b2j.run_bass_via_pjrt = run_bass_via_pjrt
    b2j._knn_patch_installed = True


_install_pjrt_patch()


def _split_sync_waits(nc, max_waits=_MAX_WAITS):
    for f in nc.m.functions:
        for bb in f.blocks:
            changed = False
            new_insts = []
            for inst in bb.instructions:
                si = inst.sync_info
                waits = list(si.on_wait) if (si and si.on_wait) else []
                if len(waits) > max_waits:
                    keep = waits[-max_waits:]
                    extra = waits[:-max_waits]
                    for j in range(0, len(extra), max_waits):
                        new_insts.append(
                            _mk_wait_carrier(inst.engine, extra[j : j + max_waits])
                        )
                    si.on_wait = keep
                    changed = True
                new_insts.append(inst)
            if changed:
                bb.instructions = new_insts


# ---------------------------------------------------------------------------
# Bass program (identical on all 8 cores; per-core data via in_maps)
# ---------------------------------------------------------------------------
def _build_program(n_strokes=S_PER_CORE, split_waits=True, mode="full"):
    nc = bass.Bass(target_bir_lowering=False, trn_type="TRN2")
    pts = n_strokes * L
    cols = pts * KOUT
    x_in = nc.dram_tensor("x_shard", [pts, D], F32, kind="ExternalInput")
    row1_in = nc.dram_tensor("row1_const", [cols], U32, kind="ExternalInput")
    base_in = nc.dram_tensor("base_col", [128, 1], F32, kind="ExternalInput")
    ident_in = nc.dram_tensor("ident_c", [128, 128], F32, kind="ExternalInput")
    negident_in = nc.dram_tensor("negident_c", [128, 128], F32, kind="ExternalInput")
    ones3_in = nc.dram_tensor("ones3_c", [3, 128], BF16, kind="ExternalInput")
    edges = nc.dram_tensor("edges", [2, cols], U32, kind="ExternalOutput")

    with TileContext(nc) as tc:
        with (
            tc.tile_pool(name="const", bufs=1) as constp,
            tc.tile_pool(name="xt", bufs=2) as xtp,
            tc.tile_pool(name="xn", bufs=3) as xnp,
            tc.tile_pool(name="big", bufs=3) as bigp,
            tc.tile_pool(name="v1p", bufs=2) as v1p,
            tc.tile_pool(name="tiny", bufs=4) as tiny,
            tc.tile_pool(name="outp", bufs=2) as outp,
            tc.tile_pool(name="ps_t", bufs=2, space="PSUM") as ps_t,
            tc.tile_pool(name="ps_b", bufs=1, space="PSUM") as ps_b,
            tc.tile_pool(name="ps_v", bufs=2, space="PSUM") as ps_v,
        ):
            ident = constp.tile([128, 128], F32)
            nc.sync.dma_start(out=ident, in_=ident_in[:, :])
            negident = constp.tile([128, 128], F32)
            nc.sync.dma_start(out=negident, in_=negident_in[:, :])
            base_col = constp.tile([128, 1], F32)
            nc.sync.dma_start(out=base_col, in_=base_in[:, :])
            ones3 = constp.tile([3, 128], BF16)
            nc.sync.dma_start(out=ones3, in_=ones3_in[:, :])

            # row 1: centers repeated, precomputed on host (2D view keeps
            # each descriptor under the 64KB SDMA limit)
            nc.sync.dma_start(
                out=edges[1, :].rearrange("(a b) -> a b", b=2304),
                in_=row1_in[:].rearrange("(a b) -> a b", b=2304),
            )

            edges_r0 = edges[0, :].rearrange(
                "(s t p j) -> s p t j", s=n_strokes, t=8, p=128, j=KOUT
            )

            last_xt_copy = None
            stroke_ctx = {}
            pending = [None]

            def flush_pending():
                if pending[0] is None:
                    return
                ps_, pt_, pv0, pmi_vals = pending[0]
                pending[0] = None
                mi_idx = tiny.tile([128, 8], U32, tag="mi_idx")
                nc.vector.max_index(mi_idx, pmi_vals, pv0)
                poutbuf, pbase = stroke_ctx[ps_]
                nc.gpsimd.iota(
                    poutbuf[:, pt_ * KOUT : pt_ * KOUT + 1],
                    pattern=[[0, 1]],
                    base=ps_ * L + pt_ * 128,
                    channel_multiplier=1,
                    allow_small_or_imprecise_dtypes=True,
                )
                nc.gpsimd.tensor_scalar_add(
                    poutbuf[:, pt_ * KOUT + 1 : (pt_ + 1) * KOUT],
                    mi_idx,
                    float(ps_ * L),
                )
                if pt_ == 7:
                    outbuf_u = outp.tile([128, 8 * KOUT], U32, tag="outbuf_u")
                    nc.gpsimd.tensor_scalar(
                        out=outbuf_u,
                        in0=poutbuf,
                        scalar1=pbase,
                        scalar2=None,
                        op0=mybir.AluOpType.add,
                    )
                    nc.sync.dma_start(
                        out=edges_r0[ps_],
                        in_=outbuf_u.rearrange("p (t j) -> p t j", j=KOUT),
                    )

            for s in range(n_strokes):
                # ---- per-stroke prep: XT chunks, sq, bias rows ----
                xt0 = xtp.tile([128, L], F32, tag="xt0")
                xt1 = xtp.tile([128, L], F32, tag="xt1")
                sqrow_ps = ps_b.tile([1, L], F32, tag="sqps")
                sqcols = tiny.tile([128, 8], F32, tag="sqcols")
                for t in range(8):
                    xn = xnp.tile([128, D], F32, tag="xn")
                    nc.sync.dma_start(
                        out=xn, in_=x_in[(s * 8 + t) * 128 : (s * 8 + t + 1) * 128, :]
                    )
                    sq_scr = xnp.tile([128, D], F32, tag="sqscr")
                    sqcol = sqcols[:, t : t + 1]
                    nc.scalar.activation(
                        sq_scr,
                        xn,
                        mybir.ActivationFunctionType.Square,
                        accum_out=sqcol,
                    )
                    for c in range(2):
                        ps = ps_t.tile([128, 128], F32, tag="tp")
                        tr = nc.tensor.transpose(
                            ps, xn[:, c * 128 : (c + 1) * 128], ident
                        )
                        if last_xt_copy is not None:
                            # Tile misses the cross-engine WAR wait when this
                            # transpose reuses a psum slot an ACT copy is
                            # still reading; ACT is in-order, so depending on
                            # the most recent copy covers all prior ones.
                            add_dep_helper(
                                tr.ins, last_xt_copy.ins, reason="ps_t WAR"
                            )
                        xt = xt0 if c == 0 else xt1
                        last_xt_copy = nc.scalar.copy(
                            xt[:, t * 128 : (t + 1) * 128], ps
                        )
                    # sqrow_ps[0, t*128+p] = sq[t*128+p]
                    nc.tensor.transpose(
                        sqrow_ps[:, t * 128 : (t + 1) * 128], sqcol, ident
                    )
                # b = -0.5*sq as a free-dim row; decompose into three bf16
                # rows (hi/mid/lo) so the bias can ride a K=3 bf16 matmul in
                # the V0 accumulation group (pure-PE psum writes only).
                brow = tiny.tile([1, L], F32, tag="brow")
                nc.scalar.activation(
                    brow, sqrow_ps, mybir.ActivationFunctionType.Copy, scale=-0.5
                )
                biash = bigp.tile([3, L], BF16, tag="biash")
                scr_f = tiny.tile([1, L], F32, tag="scr_f")
                res_f = tiny.tile([1, L], F32, tag="res_f")
                bh0 = tiny.tile([1, L], BF16, tag="bh0")
                bh1 = tiny.tile([1, L], BF16, tag="bh1")
                bh2 = tiny.tile([1, L], BF16, tag="bh2")
                bh = [bh0, bh1, bh2]
                nc.scalar.copy(bh[0], brow)
                nc.scalar.copy(scr_f, bh[0])
                nc.gpsimd.tensor_sub(res_f, brow, scr_f)
                nc.scalar.copy(bh[1], res_f)
                nc.scalar.copy(scr_f, bh[1])
                nc.gpsimd.tensor_sub(res_f, res_f, scr_f)
                nc.scalar.copy(bh[2], res_f)
                for i in range(3):
                    nc.sync.dma_start(out=biash[i : i + 1, :], in_=bh[i])

                outbuf = outp.tile([128, 8 * KOUT], F32, tag="outbuf")
                stroke_ctx[s] = (outbuf, base_col)

                # ---- per row-tile: matmul + topk ----
                for t in range(8):
                    v0ps = ps_v.tile([128, L], F32, tag="v0ps")
                    for h in range(2):
                        nc.tensor.matmul(
                            v0ps[:, h * 512 : (h + 1) * 512],
                            lhsT=ones3,
                            rhs=biash[:, h * 512 : (h + 1) * 512],
                            start=True,
                            stop=False,
                            skip_group_check=True,
                        )
                        for c, xt in enumerate((xt0, xt1)):
                            nc.tensor.matmul(
                                v0ps[:, h * 512 : (h + 1) * 512],
                                lhsT=xt[:, t * 128 : (t + 1) * 128],
                                rhs=xt[:, h * 512 : (h + 1) * 512],
                                start=False,
                                stop=False,
                                skip_group_check=True,
                            )
                    nc.tensor.matmul(
                        v0ps[:, t * 128 : (t + 1) * 128],
                        lhsT=negident,
                        rhs=ident,
                        start=False,
                        stop=True,
                        skip_group_check=True,
                    )

                    v0 = bigp.tile([128, L], F32, tag="v0")
                    nc.scalar.copy(v0, v0ps)

                    if mode == "mm_only":
                        continue
                    r1v = tiny.tile([128, 8], F32, tag="r1v")
                    nc.vector.max(out=r1v, in_=v0)
                    if mode == "no_mi":
                        continue
                    v1 = v1p.tile([128, L], F32, tag="v1")
                    nc.vector.match_replace(
                        out=v1, in_to_replace=r1v, in_values=v0, imm_value=NEG_INF
                    )
                    r2v = tiny.tile([128, 8], F32, tag="r2v")
                    nc.vector.max(out=r2v, in_=v1)
                    if mode == "no_mi3":
                        continue
                    # pack the 8 needed even-rank values on ACT (cheap HW
                    # engine, keeps the Q7 out of the DVE critical path)
                    mi_vals = tiny.tile([128, 8], F32, tag="mi_vals")
                    nc.scalar.copy(mi_vals[:, 0:4], r1v[:, 1:8:2])
                    nc.scalar.copy(mi_vals[:, 4:8], r2v[:, 1:8:2])
                    # defer this tile's max_index by one tile: the pack gets a
                    # whole tile of slack before DVE needs it
                    flush_pending()
                    pending[0] = (s, t, v0, mi_vals)
            flush_pending()

    if split_waits:
        _split_sync_waits(nc)
    return nc


_NC_CACHE = None


def _get_program():
    global _NC_CACHE
    if _NC_CACHE is None:
        _NC_CACHE = _build_program()
    return _NC_CACHE


def kernel(**inputs: np.ndarray) -> np.ndarray:
    x = np.ascontiguousarray(np.asarray(inputs["x"], dtype=np.float32))
    assert x.shape == (S * L, D), x.shape

    nc = _get_program()
    in_maps = _in_maps_for(x)
    res = run_bass_kernel_spmd(nc, in_maps, list(range(N_CORES)))
    out = np.concatenate(
        [res.results[c]["edges"] for c in range(N_CORES)], axis=1
    )
    return out.astype(np.int32)


def _in_maps_for(x):
    centers = np.arange(S * L, dtype=np.uint32)
    row1_full = np.repeat(centers, KOUT)
    in_maps = []
    for c in range(N_CORES):
        in_maps.append(
            {
                "x_shard": np.ascontiguousarray(
                    x[c * PTS_PER_CORE : (c + 1) * PTS_PER_CORE, :]
                ),
                "row1_const": row1_full[
                    c * COLS_PER_CORE : (c + 1) * COLS_PER_CORE
                ],
                "base_col": np.full((128, 1), c * PTS_PER_CORE, dtype=np.float32),
                "ident_c": np.eye(128, dtype=np.float32),
                "negident_c": (NEG_BIG * np.eye(128)).astype(np.float32),
                "ones3_c": np.ones((3, 128), dtype=ml_dtypes.bfloat16),
            }
        )
    return in_maps


def _timed_runner(nc, in_maps, iters):
    """Median wall-clock ns per execution of the sharded NEFF."""
    import time

    import jax
    from jax.sharding import Mesh, NamedSharding, PartitionSpec

    try:
        from jax.experimental.shard_map import shard_map
    except ImportError:
        from jax.shard_map import shard_map
    from concourse import bass2jax as b2j

    b2j.install_neuronx_cc_hook()
    n_cores = len(in_maps)
    partition_name = nc.partition_id_tensor.name if nc.partition_id_tensor else None
    in_names, out_names, out_avals, zero_outs = [], [], [], []
    for alloc in nc.m.functions[0].allocations:
        if not isinstance(alloc, mybir.MemoryLocationSet):
            continue
        name = alloc.memorylocations[0].name
        if alloc.kind == "ExternalInput":
            if name != partition_name:
                in_names.append(name)
        elif alloc.kind == "ExternalOutput":
            shape = list(alloc.tensor_shape)
            np_dtype = mybir.dt.np(alloc.dtype)
            out_names.append(name)
            out_avals.append(jax.core.ShapedArray(tuple(shape), np_dtype))
            zero_outs.append(np.zeros(shape, np_dtype))
    n_params = len(in_names)
    n_outs = len(out_avals)
    all_names = in_names + out_names
    if partition_name is not None:
        all_names = all_names + [partition_name]

    def _body(*args):
        operands = list(args)
        if partition_name is not None:
            operands.append(b2j.partition_id_tensor())
        outs = b2j._bass_exec_p.bind(
            *operands,
            out_avals=tuple(out_avals),
            in_names=tuple(all_names),
            out_names=tuple(out_names),
            lowering_input_output_aliases=(),
            sim_require_finite=True,
            sim_require_nnan=True,
            nc=nc,
        )
        return tuple(outs)

    devices = jax.devices()[:n_cores]
    mesh = Mesh(np.asarray(devices), ("core",))
    spec = PartitionSpec("core")
    sharded = jax.jit(
        shard_map(
            _body,
            mesh=mesh,
            in_specs=(spec,) * (n_params + n_outs),
            out_specs=(spec,) * n_outs,
            check_rep=False,
        ),
        donate_argnums=tuple(range(n_params, n_params + n_outs)),
        keep_unused=True,
    )
    shd = NamedSharding(mesh, spec)
    concat_in = [
        jax.device_put(
            np.concatenate(
                [np.asarray(in_maps[c][nm]) for c in range(n_cores)], axis=0
            ),
            shd,
        )
        for nm in in_names
    ]
    concat_zeros = [
        np.zeros((n_cores * z.shape[0], *z.shape[1:]), z.dtype) for z in zero_outs
    ]

    def one_call():
        zs = [jax.device_put(z, shd) for z in concat_zeros]
        jax.block_until_ready(zs)
        t0 = time.perf_counter()
        out = sharded(*concat_in, *zs)
        jax.block_until_ready(out)
        return time.perf_counter() - t0

    one_call()  # warmup / compile
    one_call()
    times = [one_call() for _ in range(iters)]
    times.sort()
    return times[len(times) // 2] * 1e9


def measure_exec_ns(x, iters=30):
    x = np.ascontiguousarray(np.asarray(x, dtype=np.float32))
    return _timed_runner(_get_program(), _in_maps_for(x), iters)


_NULL_NC = None


def measure_null_ns(iters=30):
    """Dispatch overhead baseline: a bass program that just copies 128B."""
    global _NULL_NC
    if _NULL_NC is None:
        nc = bass.Bass(target_bir_lowering=False, trn_type="TRN2")
        a = nc.dram_tensor("a", [1, 32], F32, kind="ExternalInput")
        b = nc.dram_tensor("b", [1, 32], F32, kind="ExternalOutput")
        with TileContext(nc) as tc:
            with tc.tile_pool(name="p", bufs=1) as pool:
                t = pool.tile([1, 32], F32)
                nc.sync.dma_start(out=t, in_=a[:, :])
                nc.sync.dma_start(out=b[:, :], in_=t)
        _split_sync_waits(nc)
        _NULL_NC = nc
    in_maps = [{"a": np.zeros((1, 32), np.float32)} for _ in range(N_CORES)]
    return _timed_runner(_NULL_NC, in_maps, iters)


if __name__ == "__main__":
    rng = np.random.default_rng(0)
    x = rng.standard_normal((S * L, D), dtype=np.float32)
    e = kernel(x=x, batch=np.zeros(S * L, np.int64), sketch_stroke_num=np.full(S, L, np.int64))
    print(e.shape, e.dtype)
    print(e[:, :12])
